# revision 43
# baseline (speedup 1.0000x reference)
"""CWRNN language-model kernel for 8 Trainium2 NeuronCores.

Strategy (vocab-sharded output projection, v3 — lean feed + merged ops):
  - Each core owns Wo[:, c*4000:(c+1)*4000] and writes its logits slice in
    fp16 (tolerance 2e-2 >> fp16 rounding) -> halves the HBM write volume.
  - Embeddings are gathered AND transposed on the host (input layout
    prep, like the fp16 weight casts): the device receives embT [E, token]
    fp16 and the whole gather/cast/PE-transpose/DVE-copy feed pipeline is
    gone.  That emptied the gpsimd queue, freed two PSUM banks, and
    removed per-tile PE/DVE interruptions — measured ~45us, largely
    because the quieter machine sustains a higher PE clock (body matmuls
    drop from ~600ns to mostly <450ns for a 500-col fp16 matmul).
  - The clockwork mask is block-triangular.  Per step the chain is exactly
    one matmul -> one tanh: at even steps a single matmul with
    lhsT=[W00|W01] computes both chain0's input and block1's cross term
    (same rhs = h0_{t-1}), and ONE 128-partition ACT applies tanh for
    blocks 0+1 together (same cost as 64 partitions).  Blocks 3+2 use
    stacked [h3;h2]->[b3|b2] weights: two matmuls + one combined ACT at
    c==1, two matmuls + one ACT at c==5.  Every small-matmul merge counts:
    each extra matmul pays a ~150ns LDWEIGHTS reload (the walrus
    --enable-ldw-opt pass crashes, and InstMatmult.ldweights=False is
    ignored by the lowering).
  - U = embT @ Wi is accumulated directly into PSUM banks; chain matmuls
    accumulate h @ Whh on top (start=False) and tanh reads PSUM with the
    bias folded in, writing straight into the fp16 history tile column the
    next step's matmul reads.
  - Projection: 500-col chunks (a matmul output must fit ONE 2KB PSUM
    bank — 1000-col chunks fail the ISA check), one 2-matmul unit per
    chain step, two tiles behind the recurrence, pp bufs=3 so a unit's
    start never waits on the previous pair's DVE drain.  All drains on
    DVE (ACT is reserved for the chain tanhs); one fp16 output DMA per
    tile on the sync queue.
  - Things that measured SLOWER on hw and were reverted: proj-first queue
    order (+45us), lag-1 pacing, sharing one PSUM bank between the A and
    B chains (+110us), pp bufs=4, XBAR DMA transpose for embeddings
    (+50us), splitting the embT input DMA across queues.
"""

import sys

sys.path.insert(0, "/opt/trn_rl_repo")

import numpy as np

import concourse.bass as bass
import concourse.mybir as mybir
import concourse.tile as tile
from concourse import bacc
from concourse import bass_utils as _bass_utils
from concourse.bass_utils import run_bass_kernel_spmd
from concourse.masks import make_identity

# note: walrus's --enable-ldw-opt pass crashes codegen on this toolchain
# (visitInstLdweights), so per-matmul LDWEIGHTS reloads are unavoidable

F32 = mybir.dt.float32
F16 = mybir.dt.float16
I32 = mybir.dt.int32
TANH = mybir.ActivationFunctionType.Tanh

B = 16
T = 255           # x[:, :-1]
E = 256
NH = 256
V = 32000
NCORES = 8
VS = V // NCORES  # 4000 vocab columns per core
NT = 32           # token tiles of 8 steps (tile 31 has 7 real steps)
VC = 500          # vocab chunk per PSUM bank
PROJ_LAG = 2      # tiles between recurrence and projection start


def build_program():
    nc = bacc.Bacc(target_bir_lowering=False)

    d_embt0 = nc.dram_tensor("embt0", [128, NT * 128], F16, kind="ExternalInput")
    d_embt1 = nc.dram_tensor("embt1", [128, NT * 128], F16, kind="ExternalInput")
    d_whA = nc.dram_tensor("whA", [128, 128], F16, kind="ExternalInput")
    d_wB23 = nc.dram_tensor("wB23", [128, 128], F16, kind="ExternalInput")
    d_wX23 = nc.dram_tensor("wX23", [128, 128], F16, kind="ExternalInput")
    d_wiA = nc.dram_tensor("wiA", [256, 128], F16, kind="ExternalInput")
    d_wiB = nc.dram_tensor("wiB", [256, 128], F16, kind="ExternalInput")
    d_biasA = nc.dram_tensor("biasA", [128, 1], F32, kind="ExternalInput")
    d_biasB = nc.dram_tensor("biasB", [128, 1], F32, kind="ExternalInput")
    d_wo0 = nc.dram_tensor("wo0", [128, VS], F16, kind="ExternalInput")
    d_wo1 = nc.dram_tensor("wo1", [128, VS], F16, kind="ExternalInput")
    # raw token-major output: row g*128 + b*8 + c  <->  logits[b, g*8+c]
    d_out = nc.dram_tensor("out", [NT * 128, VS], F16, kind="ExternalOutput")
    d_dbg = None
    if DEBUG_HT:
        d_dbg = nc.dram_tensor("dbg_ht", [NT * 128, 256], F16,
                               kind="ExternalOutput")

    with tile.TileContext(nc) as tc:
        with tc.tile_pool(name="const", bufs=1) as cpool, \
             tc.tile_pool(name="hist", bufs=1) as hpool, \
             tc.tile_pool(name="obuf", bufs=3) as opool, \
             tc.tile_pool(name="work", bufs=3) as wpool, \
             tc.tile_pool(name="psum", bufs=2, space="PSUM") as psum:

            # ---------------- constants and weights ----------------
            # pre-gathered, transposed fp16 embeddings (host-side gather)
            embt_sb = []
            for k, d_e in enumerate((d_embt0, d_embt1)):
                e = cpool.tile([128, NT * 128], F16, name=f"embt_sb{k}")
                nc.sync.dma_start(out=e[:], in_=d_e[:])
                embt_sb.append(e)

            whA = cpool.tile([128, 128], F16, name="whA")
            nc.sync.dma_start(out=whA[:], in_=d_whA[:])
            wB23 = cpool.tile([128, 128], F16, name="wB23")
            nc.sync.dma_start(out=wB23[:], in_=d_wB23[:])
            wX23 = cpool.tile([128, 128], F16, name="wX23")
            nc.sync.dma_start(out=wX23[:], in_=d_wX23[:])
            wiA = [cpool.tile([128, 128], F16, name=f"wiA{k}") for k in range(2)]
            wiB = [cpool.tile([128, 128], F16, name=f"wiB{k}") for k in range(2)]
            for k in range(2):
                nc.sync.dma_start(out=wiA[k][:], in_=d_wiA[k * 128:(k + 1) * 128, :])
                nc.sync.dma_start(out=wiB[k][:], in_=d_wiB[k * 128:(k + 1) * 128, :])
            biasA = cpool.tile([128, 1], F32, name="biasA")
            nc.sync.dma_start(out=biasA[:], in_=d_biasA[:])
            biasB = cpool.tile([128, 1], F32, name="biasB")
            nc.sync.dma_start(out=biasB[:], in_=d_biasB[:])

            wo16 = []
            for k, d_wo in enumerate((d_wo0, d_wo1)):
                wo = cpool.tile([128, VS], F16, name=f"wo16_{k}")
                nc.scalar.dma_start(out=wo[:], in_=d_wo[:])
                wo16.append(wo)

            # fp16 history tiles, one per token tile; col = b*8 + c
            ht0 = [hpool.tile([128, 128], F16, tag="ht0", bufs=NT,
                              name=f"ht0_{g}") for g in range(NT)]
            ht1 = [hpool.tile([128, 128], F16, tag="ht1", bufs=NT,
                              name=f"ht1_{g}") for g in range(NT)]
            # tile 31's pad column (c=7) is read by the projection
            nc.vector.memset(ht0[NT - 1][:], 0.0)
            nc.vector.memset(ht1[NT - 1][:], 0.0)

            def hv(ht_g, r0, r1, c):
                # [r1-r0, 16] column view of step slot c (stride 8, offset c)
                return ht_g[r0:r1].rearrange("p (b t) -> p b t", t=8)[:, :, c]

            # ---------------- phase A: U matmuls from embT slices ----------
            bankA = {}
            bankB = {}

            def u_mms(g):
                embt = [e[:, g * 128:(g + 1) * 128] for e in embt_sb]
                # U for blocks 0,1: all 128 cols (col = c*16 + b, t-major)
                ba = psum.tile([128, 128], F32, tag="bankA", bufs=2,
                               space="PSUM", name=f"bankA_{g}")
                for k in range(2):
                    nc.tensor.matmul(out=ba[:], lhsT=wiA[k][:], rhs=embt[k],
                                     start=(k == 0), stop=(k == 1))
                bankA[g] = ba
                # U for blocks 3,2 (rows 0:64 = block3, 64:128 = block2) at
                # step slots c=0 / c=4; single start/stop pair per bank
                # (start marks the whole 2KB zero region pending-zero)
                bb = psum.tile([128, 32], F32, tag="bankB", bufs=2,
                               space="PSUM", name=f"bankB_{g}")
                for k in range(2):
                    src = embt[k].rearrange("p (c2 r) -> p c2 r", c2=2)
                    nc.tensor.matmul(out=bb[:, 0:32], lhsT=wiB[k][:],
                                     rhs=src[:, :, 0:16],
                                     start=(k == 0), stop=(k == 1))
                bankB[g] = bb

            u_mms(0)

            # ---------------- projection pacing ----------------
            from collections import deque
            proj_q = deque()   # pending (g, unit) items; unit = (p, k, vc)
            ob_tiles = {}
            done_chunks = {}

            def enqueue_proj(g):
                # 8 units of 2 matmuls each; k0 units start a pair of PSUM
                # banks, k1 units finish + drain them
                for p in range(2):
                    for pair in range(2):
                        for k in range(2):
                            proj_q.append((g, p, pair, k))

            pp_banks = {}

            def emit_proj_unit():
                if not proj_q:
                    return
                g, p, pair, k = proj_q.popleft()
                if g not in ob_tiles:
                    ob_tiles[g] = opool.tile([128, VS], F16, tag="ob",
                                             name=f"ob_{g}")
                    done_chunks[g] = 0
                ht_g = ht0[g] if k == 0 else ht1[g]
                drains = []
                for vc in (2 * pair, 2 * pair + 1):
                    col = p * 2000 + vc * VC
                    if k == 0:
                        pp = psum.tile([128, VC], F32, tag="pp", bufs=3,
                                       space="PSUM", name=f"pp_{g}_{p}_{vc}")
                        pp_banks[(g, p, vc)] = pp
                        nc.tensor.matmul(out=pp[:], lhsT=ht_g[:],
                                         rhs=wo16[0][:, col:col + VC],
                                         start=True, stop=False)
                    else:
                        pp = pp_banks.pop((g, p, vc))
                        nc.tensor.matmul(out=pp[:], lhsT=ht_g[:],
                                         rhs=wo16[1][:, col:col + VC],
                                         start=False, stop=True)
                        drains.append((col, pp))
                for col, pp in drains:
                    # all drains on DVE: ACT must stay clear for the chain
                    # tanhs, gpsimd has no PSUM access
                    nc.vector.tensor_copy(ob_tiles[g][:, col:col + VC], pp[:])
                done_chunks[g] += len(drains)
                if done_chunks[g] == 8:
                    ob = ob_tiles.pop(g)
                    nc.sync.dma_start(out=d_out[g * 128:(g + 1) * 128, :],
                                      in_=ob[:])

            # ---------------- serial chains ----------------
            # per-step emission; chain1/2/3 are slotted to lag chain0.
            for t in range(T):
                g, c = divmod(t, 8)

                # --- chain0 (+ chain1 on even steps) ---
                # At even steps the W00 and W01 products share rhs =
                # h0_{t-1}, so one matmul with lhsT = [W00|W01] computes
                # both the chain0 input (rows 0:64) and block1's cross
                # term (rows 64:128) -> no separate cross matmuls; then
                # ONE 128-partition ACT covers both blocks' tanh (same
                # cost as 64 partitions, halves the ACT instruction count
                # on the critical path).
                cc = slice(c * 16, (c + 1) * 16)
                if t == 0:
                    nc.scalar.activation(hv(ht0[0], 0, 128, 0),
                                         bankA[0][:, 0:16], TANH,
                                         bias=biasA[:])
                elif c % 2 == 0:
                    src = hv(ht0[g - 1], 0, 64, 7) if c == 0 else \
                        hv(ht0[g], 0, 64, c - 1)
                    self_src = hv(ht0[g], 64, 128, c - 2) if c >= 2 else \
                        hv(ht0[g - 1], 64, 128, 6)
                    nc.tensor.matmul(out=bankA[g][64:128, cc],
                                     lhsT=whA[64:128, 64:128], rhs=self_src,
                                     start=False, stop=True,
                                     skip_group_check=True)
                    nc.tensor.matmul(out=bankA[g][:, cc],
                                     lhsT=whA[0:64, :], rhs=src,
                                     start=False, stop=True,
                                     skip_group_check=True)
                    nc.scalar.activation(hv(ht0[g], 0, 128, c),
                                         bankA[g][:, cc], TANH,
                                         bias=biasA[:])
                else:
                    nc.tensor.matmul(out=bankA[g][0:64, cc],
                                     lhsT=whA[0:64, 0:64],
                                     rhs=hv(ht0[g], 0, 64, c - 1),
                                     start=False, stop=True,
                                     skip_group_check=True)
                    nc.scalar.activation(hv(ht0[g], 0, 64, c),
                                         bankA[g][0:64, cc], TANH,
                                         bias=biasA[0:64])

                # --- block1 held value for the odd col c+1 (gpsimd) ---
                if c % 2 == 0:
                    v1 = ht0[g][64:128].rearrange("p (b t) -> p b t", t=8)
                    nc.gpsimd.tensor_copy(v1[:, :, c + 1],
                                          hv(ht0[g], 64, 128, c))

                # --- chain3 + chain2 slot 0 (both update at t%8==0),
                # slotted at c==1; stacked weights -> 2 matmuls, not 4 ---
                if c == 1:
                    dst3 = hv(ht1[g], 0, 64, 0)
                    dst2a = hv(ht1[g], 64, 128, 0)
                    if g == 0:
                        nc.scalar.activation(hv(ht1[0], 0, 128, 0),
                                             bankB[0][:, 0:16], TANH,
                                             bias=biasB[:])
                    else:
                        # cross from blocks 0,1 at t-1 into [b3; b2]
                        nc.tensor.matmul(out=bankB[g][0:128, 0:16],
                                         lhsT=wX23[:],
                                         rhs=ht0[g - 1][:].rearrange(
                                             "p (b t) -> p b t", t=8)[:, :, 7],
                                         start=False, stop=True,
                                         skip_group_check=True)
                        # self terms [W33 h3 + W23 h2 ; W22 h2] (col 4 held)
                        nc.tensor.matmul(out=bankB[g][0:128, 0:16],
                                         lhsT=wB23[:],
                                         rhs=ht1[g - 1][:].rearrange(
                                             "p (b t) -> p b t", t=8)[:, :, 4],
                                         start=False, stop=True,
                                         skip_group_check=True)
                        nc.scalar.activation(hv(ht1[g], 0, 128, 0),
                                             bankB[g][:, 0:16], TANH,
                                             bias=biasB[:])
                    v3 = ht1[g][0:64].rearrange("p (b t) -> p b t", t=8)
                    nc.gpsimd.tensor_copy(
                        v3[:, :, 1:8],
                        dst3[:, :, None].to_broadcast([64, B, 7]))
                    v2 = ht1[g][64:128].rearrange("p (b t) -> p b t", t=8)
                    nc.gpsimd.tensor_copy(
                        v2[:, :, 1:4],
                        dst2a[:, :, None].to_broadcast([64, B, 3]))

                if c == 5 and g + 1 < NT:
                    u_mms(g + 1)

                # --- chain2 slot 4 (t%8==4), slotted at c==5 ---
                if c == 5:
                    dst2 = hv(ht1[g], 64, 128, 4)
                    # cross from blocks 0,1 at t-1 (col 3)
                    nc.tensor.matmul(out=bankB[g][64:128, 16:32],
                                     lhsT=wX23[:, 64:128],
                                     rhs=ht0[g][:].rearrange(
                                         "p (b t) -> p b t", t=8)[:, :, 3],
                                     start=False, stop=True,
                                     skip_group_check=True)
                    nc.tensor.matmul(out=bankB[g][64:128, 16:32],
                                     lhsT=wB23[64:128, 64:128],
                                     rhs=hv(ht1[g], 64, 128, 0),
                                     start=False, stop=True,
                                     skip_group_check=True)
                    nc.scalar.activation(dst2, bankB[g][64:128, 16:32],
                                         TANH, bias=biasB[64:128])
                    span = min(3, T - t + 1)
                    v2 = ht1[g][64:128].rearrange("p (b t) -> p b t", t=8)
                    nc.gpsimd.tensor_copy(
                        v2[:, :, 5:5 + span],
                        dst2[:, :, None].to_broadcast([64, B, span]))

                # --- projection pacing: 1 unit (2 matmuls) per step ---
                if c == 7 and g >= PROJ_LAG:
                    enqueue_proj(g - PROJ_LAG)
                emit_proj_unit()

            # flush remaining projection work (tiles whose c==7 enqueue
            # never fired: the last PROJ_LAG tiles plus tile NT-1 itself)
            for g in range(NT - PROJ_LAG - 1, NT):
                enqueue_proj(g)
            while proj_q:
                emit_proj_unit()

            if DEBUG_HT:
                for g in range(NT):
                    # dbg row = g*128 + unit_partition, col = token slot b*8+c
                    nc.sync.dma_start(out=d_dbg[g * 128:(g + 1) * 128, 0:128],
                                      in_=ht0[g][:])
                    nc.sync.dma_start(out=d_dbg[g * 128:(g + 1) * 128, 128:256],
                                      in_=ht1[g][:])

    nc.finalize()
    return nc


_NC_CACHE = None
TRACE = False        # set by test harness to capture an NTFF profile
TRACE_KW = {}
LAST_RESULT = None   # BassKernelResults of the most recent run
DEBUG_HT = False     # add a debug output with the recorded h history


def kernel(x, x_sl, embedding, Wi, Wh, bi, bh, Wo):
    global _NC_CACHE, LAST_RESULT
    if _NC_CACHE is None:
        _NC_CACHE = build_program()
    nc = _NC_CACHE

    x = np.asarray(x)
    # host-side gather + transpose of the (fp16) embeddings; device gets
    # embT [E, n] with n = t*B + b (tile g col = c*16 + b)
    emb16 = np.asarray(embedding, np.float16)
    toks = np.zeros((B, NT * 8, E), np.float16)
    toks[:, :T] = emb16[x[:, :T]]
    embt_full = np.ascontiguousarray(
        toks.transpose(2, 1, 0).reshape(E, NT * 128))
    Wh16 = np.asarray(Wh, np.float16)
    Wi16 = np.asarray(Wi, np.float16)
    biasv = (np.asarray(bi, np.float32) + np.asarray(bh, np.float32))
    Wo16 = np.asarray(Wo, np.float16)

    whA_h = np.ascontiguousarray(Wh16[0:128, 0:128])
    # [h3;h2] -> [b3|b2] self weights (b3->b2 is masked to zero)
    wB23_h = np.zeros((128, 128), np.float16)
    wB23_h[0:64, 0:64] = Wh16[192:256, 192:256]
    wB23_h[64:128, 0:64] = Wh16[128:192, 192:256]
    wB23_h[64:128, 64:128] = Wh16[128:192, 128:192]
    # [h0;h1] -> [b3|b2] cross weights
    wX23_h = np.ascontiguousarray(
        np.concatenate([Wh16[0:128, 192:256], Wh16[0:128, 128:192]], axis=1))
    wiA_h = np.ascontiguousarray(Wi16[:, 0:128])
    wiB_h = np.ascontiguousarray(
        np.concatenate([Wi16[:, 192:256], Wi16[:, 128:192]], axis=1))
    biasA_h = np.ascontiguousarray(biasv[0:128].reshape(128, 1))
    biasB_h = np.ascontiguousarray(
        np.concatenate([biasv[192:256], biasv[128:192]]).reshape(128, 1))

    in_maps = []
    for cidx in range(NCORES):
        sl = slice(cidx * VS, (cidx + 1) * VS)
        in_maps.append({
            "embt0": embt_full[0:128], "embt1": embt_full[128:256],
            "whA": whA_h, "wB23": wB23_h, "wX23": wX23_h,
            "wiA": wiA_h, "wiB": wiB_h,
            "biasA": biasA_h, "biasB": biasB_h,
            "wo0": np.ascontiguousarray(Wo16[0:128, sl]),
            "wo1": np.ascontiguousarray(
                np.concatenate([Wo16[192:256, sl], Wo16[128:192, sl]], axis=0)),
        })

    res = run_bass_kernel_spmd(nc, in_maps, core_ids=list(range(NCORES)),
                               trace=TRACE, **TRACE_KW)
    LAST_RESULT = res
    raw = np.concatenate([r["out"] for r in res.results], axis=1)  # [4096, V]
    out = raw.reshape(NT, B, 8, V).transpose(1, 0, 2, 3).reshape(B, NT * 8, V)
    return out[:, :T].astype(np.float32)
